# revision 22
# baseline (speedup 1.0000x reference)
"""FFM (field-aware factorization machine) forward kernel for 8 TRN2 NeuronCores.

y[b] = x[b] @ w_lin + b_lin + sum_{i<j} Wu[i,j] x[b,i] x[b,j]
with Wu = triu(Wmat, 1), Wmat[i,j] = <v[i, field[j]], v[j, field[i]]>.

Strategy:
  - Host: build Wmat from (v, field_idx)  [tiny: 256x256x8], symmetrize
    S = (Wu + Wu^T)/2, eigendecompose S = Q diag(lam) Q^T.  Then
    x^T Wu x = sum_n lam_n (x . q_n)^2.
  - Device (data-parallel over batch, 8 cores), all-bf16 datapath
    (~0.5% rel err, 4x under the 2e-2 gate; halves HBM traffic vs fp32r):
      * z^T = Q^T X^T via bf16 matmuls (fp32 PSUM accumulate),
      * squares split across ScalarE (direct from PSUM, bf16 out) and
        VectorE (bf16 copy + 16-bit square),
      * lambda-weighted partition-reduction matmuls with a 16-wide masked
        lambda table: chunk c lands on PSUM row c, so all 16 chunks of a
        core accumulate into ONE [16, 512] tile per 128-eigencomp half.
        The two halves target PE column groups 0 and 1 (tile_position) so
        their matmuls run concurrently on hardware, landing on PSUM
        partition rows 0-15 and 32-47 of the same bank.
      * two [16, 512] copies move the result to SBUF; the halves are
        summed on the host (batch order is y[512*c + f] = row c col f).
  - Host feeds x pre-transposed ([256, B/8] per core, bf16) so the
    contraction dim lands on SBUF partitions with zero on-device
    transposes. A few dummy matmuls on the Q tile warm the PE clock
    (HAM) during the initial x DMA.
"""

import numpy as np
import ml_dtypes

_B, _N = 65536, 256
_NCORES = 8
_BS = _B // _NCORES  # 8192 batch rows per core
_FCH = 512           # matmul moving free-dim chunk (1 PSUM bank of fp32)
_NCH = _BS // _FCH   # 16 chunks per core
# chunks whose pz1 square also runs on ScalarE (load balance ACT vs DVE;
# 15 excluded so ACT and DVE split the final chunk's squares)
_ACT_EXTRA = frozenset({1, 5, 9, 12})
# x DMA column plan: small chunks first to shorten the pipeline head
_COL_PLAN = ((0, 512), (512, 512), (1024, 1024), (2048, 2048),
             (4096, 2048), (6144, 2048))

_compiled_nc = {}


def _build_nc(reps=1, act_extra=_ACT_EXTRA, dummies=6, pz_bufs=2,
              xin_bufs=4, col_plan=_COL_PLAN, const_gpsimd=True,
              tail_split=True):
    from concourse import bacc, mybir, tile

    f32 = mybir.dt.float32
    bf16 = mybir.dt.bfloat16
    Act = mybir.ActivationFunctionType

    nc = bacc.Bacc("TRN2", target_bir_lowering=False, debug=False)

    xt = nc.dram_tensor("xt", [_N, _BS], bf16, kind="ExternalInput").ap()
    q = nc.dram_tensor("q", [_N, _N], bf16, kind="ExternalInput").ap()
    # masked lambda table:
    # lam[p, half*256 + m*16 + j] = lam[half*128 + p] * (j == m)
    lam = nc.dram_tensor("lam", [128, 512], bf16, kind="ExternalInput").ap()
    # y_dram rows 0-15: eigen-half 0, rows 32-47: half 1 (16-31 unused);
    # row c, col f = partial y for batch item 512*c + f
    y = nc.dram_tensor("y", [48, _FCH], bf16, kind="ExternalOutput").ap()

    with tile.TileContext(nc) as tc:
        with (
            tc.tile_pool(name="const", bufs=1) as cpool,
            tc.tile_pool(name="xin", bufs=xin_bufs) as xpool,
            tc.tile_pool(name="zsq", bufs=3) as zpool,
            tc.tile_pool(name="yout", bufs=1) as ypool,
            tc.tile_pool(name="pz", bufs=pz_bufs, space="PSUM") as pzpool,
            tc.tile_pool(name="py", bufs=1, space="PSUM") as pypool,
            tc.tile_pool(name="warm", bufs=1, space="PSUM") as wpool,
        ):
            # PE warm-up: junk matmuls on a memset tile while x/q load.
            # (HAM needs ~3.4us of sustained PE activity to unthrottle; these
            # run during the DMA head so the real matmuls start warm.)
            if dummies:
                wtile = cpool.tile([128, 256], bf16)
                nc.gpsimd.memset(wtile[:], 0)
                scratch = wpool.tile([128, 256], f32)
                for _ in range(dummies):
                    nc.tensor.matmul(scratch[:], wtile[:, 0:128], wtile[:, :],
                                     start=True, stop=True)

            # Constants: Q split into two 128-row chunks; masked lam table.
            cdma = nc.gpsimd.dma_start if const_gpsimd else nc.sync.dma_start
            q0 = cpool.tile([128, _N], bf16)
            q1 = cpool.tile([128, _N], bf16)
            lam_sb = cpool.tile([128, 512], bf16)
            cdma(q0[:], q[0:128, :])
            cdma(q1[:], q[128:256, :])
            cdma(lam_sb[:], lam[:, :])

            # y staging: rows 0-15 = eigen-half 0, rows 32-47 = half 1
            # (16-31 unused; zeroed so the one-shot [48, 512] DMA reads
            # initialized memory)
            y_sb = ypool.tile([48, _FCH], bf16)
            nc.gpsimd.memset(y_sb[:, :], 0)

            state = {"pyA": None, "pyB": None}

            def emit_reduce(prev):
                c, zs0, zs1 = prev
                if c == 0:
                    # separate tiles (separate banks) so the y copies don't
                    # serialize on each other's reduce matmuls
                    state["pyA"] = pypool.tile([16, _FCH], f32, tag="pyA",
                                               name="pyA")
                    state["pyB"] = pypool.tile([48, _FCH], f32, tag="pyB",
                                               name="pyB")
                pyA, pyB = state["pyA"], state["pyB"]
                # lhsT col j = lam_half * (j == c): chunk c lands on row c.
                nc.tensor.matmul(pyA[0:16, :], lam_sb[:, c * 16:c * 16 + 16],
                                 zs0[:], start=(c == 0), stop=(c == _NCH - 1),
                                 tile_position=(0, 0))
                nc.tensor.matmul(pyB[32:48, :],
                                 lam_sb[:, 256 + c * 16:256 + c * 16 + 16],
                                 zs1[:], start=(c == 0), stop=(c == _NCH - 1),
                                 tile_position=(0, 32))
                if c == _NCH - 1:
                    nc.vector.tensor_copy(y_sb[0:16, :], pyA[0:16, :])
                    # ACT runs the second copy concurrently (Copy shares the
                    # act-func set with Square: no table swap)
                    nc.scalar.activation(y_sb[32:48, :], pyB[32:48, :],
                                         Act.Copy)

            prev = None
            for _rep in range(reps):
              for col0, w in col_plan:
                x0 = xpool.tile([128, w], bf16, tag=f"x0_{w}")
                x1 = xpool.tile([128, w], bf16, tag=f"x1_{w}")
                nc.sync.dma_start(x0[:], xt[0:128, col0:col0 + w])
                nc.sync.dma_start(x1[:], xt[128:256, col0:col0 + w])
                for k in range(w // _FCH):
                    c = (col0 + k * _FCH) // _FCH
                    last = c == _NCH - 1
                    pz0 = pzpool.tile([128, _FCH], f32, tag="pz0")
                    pz1 = pzpool.tile([128, _FCH], f32, tag="pz1")
                    zs0 = zpool.tile([128, _FCH], bf16, tag="zs0")
                    zs1 = zpool.tile([128, _FCH], bf16, tag="zs1")
                    # The final chunk is emitted as two 256-col segments so
                    # its square -> reduce -> copy tail chain is shorter.
                    segs = ((0, _FCH),) if not (last and tail_split) \
                        else ((0, 256), (256, 256))
                    for s0, sw in segs:
                        r0 = x0[:, k * _FCH + s0:k * _FCH + s0 + sw]
                        r1 = x1[:, k * _FCH + s0:k * _FCH + s0 + sw]
                        sl = slice(s0, s0 + sw)
                        # z^T[n, b] = sum_i Q[i, n] * xT[i, b]; x0-half pair
                        # first so chunk 0 starts before x1 lands.
                        nc.tensor.matmul(pz0[:, sl], q0[:, 0:128], r0,
                                         start=True, stop=False,
                                         skip_group_check=last)
                        nc.tensor.matmul(pz1[:, sl], q0[:, 128:256], r0,
                                         start=True, stop=False,
                                         skip_group_check=last)
                        nc.tensor.matmul(pz0[:, sl], q1[:, 0:128], r1,
                                         start=False, stop=True,
                                         skip_group_check=last)
                        nc.tensor.matmul(pz1[:, sl], q1[:, 128:256], r1,
                                         start=False, stop=True,
                                         skip_group_check=last)
                        if s0 == 0 and prev is not None:
                            # reduce the previous chunk while squares cook
                            emit_reduce(prev)
                        nc.scalar.activation(zs0[:, sl], pz0[:, sl],
                                             Act.Square)
                        if c in act_extra:
                            nc.scalar.activation(zs1[:, sl], pz1[:, sl],
                                                 Act.Square)
                        else:
                            t1 = zpool.tile([128, _FCH], bf16, tag="t1")
                            nc.vector.tensor_copy(t1[:, sl], pz1[:, sl])
                            nc.vector.tensor_mul(zs1[:, sl], t1[:, sl],
                                                 t1[:, sl])
                    prev = (c, zs0, zs1)
              emit_reduce(prev)
              prev = None

            nc.sync.dma_start(y[:, :], y_sb[:])

    nc.compile()
    return nc


def _get_nc(reps=1, **kw):
    key = (reps,) + tuple(sorted(kw.items()))
    if key not in _compiled_nc:
        _compiled_nc[key] = _build_nc(reps, **kw)
    return _compiled_nc[key]


def _host_prep(x, w_lin, b_lin, v, field_idx):
    """Host-side tiny-param preprocessing + sharding. Returns (in_maps, lin)."""
    x = np.asarray(x, dtype=np.float32)
    w_lin = np.asarray(w_lin, dtype=np.float32)
    b_lin = np.asarray(b_lin, dtype=np.float32)
    v = np.asarray(v, dtype=np.float64)
    field_idx = np.asarray(field_idx, dtype=np.int64)

    # Wmat[i, j] = <v[i, field[j]], v[j, field[i]]>
    A = v[:, field_idx, :]                       # [N, N, K]
    Wmat = np.einsum('ijk,jik->ij', A, A)        # [N, N]
    Wu = np.triu(Wmat, 1)
    S = (Wu + Wu.T) * 0.5
    lam, Q = np.linalg.eigh(S)                   # S = Q diag(lam) Q^T
    # Interleave components between the two device halves so each half's
    # partial sum is near zero (the halves are accumulated in bf16; without
    # this they are large and cancelling, amplifying rounding error).
    perm = np.concatenate([np.arange(0, _N, 2), np.arange(1, _N, 2)])
    lam, Q = lam[perm], Q[:, perm]
    Q16 = Q.astype(np.float32).astype(ml_dtypes.bfloat16)
    # masked lambda table [128, 512]:
    # col half*256 + m*16 + j = lam[half*128 + p] * (j == m)
    lam2 = lam.astype(np.float32).reshape(2, 128).T  # [p, half]
    lam_tbl = np.zeros((128, 2, _NCH, 16), dtype=np.float32)
    for m in range(_NCH):
        lam_tbl[:, :, m, m] = lam2
    lam16 = lam_tbl.reshape(128, 512).astype(ml_dtypes.bfloat16)

    # x transposed + sharded along batch, cast to bf16
    xts = x.reshape(_NCORES, _BS, _N).transpose(0, 2, 1)  # [8, N, BS]
    xts = np.ascontiguousarray(xts).astype(ml_dtypes.bfloat16)

    in_maps = [
        {"xt": xts[i], "q": Q16, "lam": lam16} for i in range(_NCORES)
    ]
    lin = x @ w_lin + b_lin[0]                   # linear part on host (0.4% of FLOPs)
    return in_maps, lin


def _unscramble(y_core):
    """[48, 512] device layout -> [8192] batch order (sums eigen halves)."""
    y_core = np.asarray(y_core, dtype=np.float32)
    return (y_core[0:16] + y_core[32:48]).reshape(_BS)


def _run_device(in_maps, trace=False, reps=1):
    from concourse.bass_utils import run_bass_kernel_spmd

    nc = _get_nc(reps)
    res = run_bass_kernel_spmd(
        nc, in_maps, core_ids=list(range(_NCORES)), trace=trace
    )
    yq = np.concatenate(
        [_unscramble(res.results[i]["y"]) for i in range(_NCORES)]
    )
    return yq, res


def kernel(x, w_lin, b_lin, v, field_idx):
    in_maps, lin = _host_prep(x, w_lin, b_lin, v, field_idx)
    yq, _ = _run_device(in_maps, trace=False)
    return (lin + yq).astype(np.float32)[:, None]
